# revision 20
# baseline (speedup 1.0000x reference)
"""Trainium2 Bass kernel for nn_BodyKDV8 (KL-divergence distillation loss).

Math (per voxel v, per batch b):
    kl[v] = sum_c q_c*(logq_c - logp_c)      q = softmax(T), p = softmax(S)
          = W/ZT + log(ZS) - log(ZT)
    where ZT = sum_c exp(T_c), ZS = sum_c exp(S_c), W = sum_c exp(T_c)*(T_c-S_c)
(no max-subtraction needed: inputs are ~N(0,1), exp stays well in range).

Device computes the three channel-sum fields ZT, W, ZS; the host finishes
with kl = W/ZT + log(ZS/ZT), then a weighted bincount over gt labels
(exactly reproducing segment_sum + masked mean -> scalar loss).

Device layout: voxels of a per-core chunk are split into G=9 groups of
GL contiguous voxels; SBUF tiles are [126, F] with partition r = g*14+c
holding channel c of voxel-group g (r traverses (g, c) lexicographically,
so DRAM views map to flat tiles). Channel sums over the 14 partitions of
each group are TensorE matmuls with a block-ones lhsT; slice k of a pack
uses lhsT_k [126, 108] with ones at [g*14+c, 9k+g], accumulating 12
slices into one PSUM bank [108, 512] so PSUM->SBUF copies and the output
DMA run at full partition utilization.

Inputs stream as fp16 (host-converted: halves HBM traffic; logits are
N(0,1) so fp16 quantization perturbs the final scalar by ~4e-6 relative).
Matmul operands are fp16 (1 col/cycle on PE vs 4 for fp32); PSUM
accumulation stays fp32 and the ZT/W/ZS outputs are returned as fp32.

Sharding: data-parallel over voxels, 8 cores, each core takes a
contiguous 1/8 slice of both batches. Scalar reduction happens on host.
"""

import numpy as np

for _p in ("/opt/trn_rl_repo", "/root/.axon_site/_ro/trn_rl_repo"):
    import sys

    if _p not in sys.path:
        sys.path.append(_p)

import concourse.bacc as bacc
import concourse.bass as bass
import concourse.tile as tile
from concourse import mybir
from concourse.bass_utils import run_bass_kernel_spmd

F32 = mybir.dt.float32
F16 = mybir.dt.float16
AF = mybir.ActivationFunctionType

B = 2
C = 14
N_TOT = 96 * 96 * 96          # 884736 voxels per batch
NCORES = 8
NC_VOX = N_TOT // NCORES      # 110592 voxels per core per batch
G = 9                         # voxel groups -> 126 = 9*14 used partitions
GL = NC_VOX // G              # 12288 voxels per group
SL = 512                      # matmul slice = one fp32 PSUM bank
K_PER_PACK = 12               # slices packed per PSUM bank (108 partitions)
PACK_F = SL * K_PER_PACK      # 6144 free-span per pack
N_PACKS = GL // PACK_F        # 2 packs per batch
QUARTERS = 2                  # loads per pack
Q_F = PACK_F // QUARTERS      # 3072 free-span per load
PACK_ROWS = G * K_PER_PACK    # 108
NQ = 3                        # ZT, W, ZS

_NC_CACHE = {}


def _build_nc():
    nc = bacc.Bacc("TRN2", target_bir_lowering=False, debug=False)

    s_dram = nc.dram_tensor("s", [B, C, NC_VOX], F16, kind="ExternalInput")
    t_dram = nc.dram_tensor("t", [B, C, NC_VOX], F16, kind="ExternalInput")
    # lhsT_k [126, 108]: ones at [g*14+c, 9k+g]
    ones_dram = nc.dram_tensor(
        "ones_blk", [126, K_PER_PACK, PACK_ROWS], F16, kind="ExternalInput"
    )
    # per (batch, pack): rows r=9k+g, then ZT|W|ZS, then 512 voxel cols
    out_dram = nc.dram_tensor(
        "zws", [B, N_PACKS, PACK_ROWS, NQ, SL], F32, kind="ExternalOutput"
    )

    s_ap = s_dram.ap()
    t_ap = t_dram.ap()
    out_ap = out_dram.ap()

    with tile.TileContext(nc) as tc:
        with (
            tc.tile_pool(name="singles", bufs=1) as singles,
            tc.tile_pool(name="io_s", bufs=3) as io_s,
            tc.tile_pool(name="io_t", bufs=3) as io_t,
            tc.tile_pool(name="es", bufs=3) as es_pool,
            tc.tile_pool(name="dd", bufs=3) as dd_pool,
            tc.tile_pool(name="et", bufs=6) as et_pool,
            tc.tile_pool(name="pp", bufs=6) as pp_pool,
            tc.tile_pool(name="psum", bufs=2, space="PSUM") as psum,
            tc.tile_pool(name="cop", bufs=2) as cop_pool,
        ):
            ones_t = singles.tile([126, K_PER_PACK, PACK_ROWS], F16)
            nc.sync.dma_start(out=ones_t[:], in_=ones_dram.ap())

            for b in range(B):
                # [C, NC_VOX] -> [G, C, GL]: partition row g*14+c <-> (g, c)
                sb = s_ap[b].rearrange("c (g f) -> g c f", g=G)
                tb = t_ap[b].rearrange("c (g f) -> g c f", g=G)

                for p in range(N_PACKS):
                    zt = psum.tile([PACK_ROWS, SL], F32, tag="zt")
                    wm = psum.tile([PACK_ROWS, SL], F32, tag="wm")
                    zs = psum.tile([PACK_ROWS, SL], F32, tag="zs")

                    for q in range(QUARTERS):
                        f0 = p * PACK_F + q * Q_F
                        s_t = io_s.tile([126, Q_F], F16)
                        t_t = io_t.tile([126, Q_F], F16)
                        nc.sync.dma_start(
                            out=s_t[:], in_=sb[:, :, f0 : f0 + Q_F]
                        )
                        nc.sync.dma_start(
                            out=t_t[:], in_=tb[:, :, f0 : f0 + Q_F]
                        )
                        # producer order tuned so PE never starves: et
                        # halves first (zt matmuls), pp next (wm), eS last
                        # (zs). Half-size et/pp tiles release PE deps early.
                        H = Q_F // 2
                        d = dd_pool.tile([126, Q_F], F16)
                        nc.vector.tensor_sub(d[:], t_t[:], s_t[:])
                        ets, pps = [], []
                        for h in range(2):
                            hc = slice(h * H, (h + 1) * H)
                            et = et_pool.tile([126, H], F16)
                            nc.scalar.activation(et[:], t_t[:, hc], AF.Exp)
                            pp = pp_pool.tile([126, H], F16)
                            nc.vector.tensor_mul(pp[:], et[:], d[:, hc])
                            ets.append(et)
                            pps.append(pp)
                        eS = es_pool.tile([126, Q_F], F16)
                        nc.scalar.activation(eS[:], s_t[:], AF.Exp)

                        nsl = Q_F // SL
                        for src, bank in ((ets, zt), (pps, wm), (None, zs)):
                            for j in range(nsl):
                                k = q * nsl + j
                                lhsT = ones_t[:, k, :]
                                if src is None:
                                    rhs = eS[:, j * SL : (j + 1) * SL]
                                else:
                                    half = src[j // (nsl // 2)]
                                    jj = j % (nsl // 2)
                                    rhs = half[:, jj * SL : (jj + 1) * SL]
                                nc.tensor.matmul(
                                    bank[:, :], lhsT, rhs,
                                    start=(k == 0), stop=(k == K_PER_PACK - 1),
                                )

                    # PSUM -> SBUF, then one contiguous 663KB writeback
                    cop = cop_pool.tile([PACK_ROWS, NQ, SL], F32)
                    nc.vector.tensor_copy(cop[:, 0, :], zt[:])
                    nc.vector.tensor_copy(cop[:, 1, :], wm[:])
                    nc.vector.tensor_copy(cop[:, 2, :], zs[:])
                    nc.sync.dma_start(out=out_ap[b, p], in_=cop[:])

    nc.compile()
    return nc


def _get_nc():
    if "nc" not in _NC_CACHE:
        _NC_CACHE["nc"] = _build_nc()
    return _NC_CACHE["nc"]


def _ones_blk():
    o = np.zeros((126, K_PER_PACK, PACK_ROWS), dtype=np.float16)
    r = np.arange(126)
    for k in range(K_PER_PACK):
        o[r, k, G * k + r // C] = 1.0
    return o


def kernel(preds_S, preds_T, gt_labels, _results_hook=None):
    S = np.asarray(preds_S, dtype=np.float16).reshape(B, C, N_TOT)
    T = np.asarray(preds_T, dtype=np.float16).reshape(B, C, N_TOT)
    labels = np.asarray(gt_labels).reshape(B, N_TOT)

    nc = _get_nc()
    ones = _ones_blk()
    in_maps = []
    for m in range(NCORES):
        sl = slice(m * NC_VOX, (m + 1) * NC_VOX)
        in_maps.append(
            {
                "s": np.ascontiguousarray(S[:, :, sl]),
                "t": np.ascontiguousarray(T[:, :, sl]),
                "ones_blk": ones,
            }
        )

    res = run_bass_kernel_spmd(nc, in_maps, list(range(NCORES)))
    if _results_hook is not None:
        _results_hook(res)

    # reassemble ZT/W/ZS into [B, N_TOT] voxel order:
    # out[b, p, 9k+g, q, v] <-> voxel (core m) m*NC_VOX + g*GL + p*PACK_F + k*SL + v
    fields = np.empty((NQ, B, N_TOT), dtype=np.float32)
    for m in range(NCORES):
        zws = res.results[m]["zws"]  # [B, N_PACKS, 108, 3, 512]
        a = zws.reshape(B, N_PACKS, K_PER_PACK, G, NQ, SL)
        # -> [NQ, B, G, N_PACKS, K_PER_PACK, SL] -> [NQ, B, NC_VOX]
        a = a.transpose(4, 0, 3, 1, 2, 5).reshape(NQ, B, NC_VOX)
        fields[:, :, m * NC_VOX : (m + 1) * NC_VOX] = a

    ZT, W, ZS = fields[0], fields[1], fields[2]
    kl = W / ZT + np.log(ZS) - np.log(ZT)

    # host finale: segment sums per (batch, class), masked mean, class 0 excluded
    loss = 0.0
    for b in range(B):
        lab = labels[b].astype(np.int64)
        sums = np.bincount(lab, weights=kl[b].astype(np.float64), minlength=C)
        counts = np.bincount(lab, minlength=C)
        terms = np.where(counts > 0, sums / (C * np.maximum(counts, 1)), 0.0)
        loss += terms[1:].sum()
    return np.float32(loss)


# revision 23
# speedup vs baseline: 1.0600x; 1.0600x over previous
"""Trainium2 Bass kernel for nn_BodyKDV8 (KL-divergence distillation loss).

Math (per voxel v, per batch b):
    kl[v] = sum_c q_c*(logq_c - logp_c)      q = softmax(T), p = softmax(S)
          = W/ZT + log(ZS) - log(ZT)
    where ZT = sum_c exp(T_c), ZS = sum_c exp(S_c), W = sum_c exp(T_c)*(T_c-S_c)
(no max-subtraction needed: inputs are ~N(0,1), exp stays well in range).

Device computes the three channel-sum fields ZT, W, ZS; the host finishes
with kl = W/ZT + log(ZS/ZT), then a weighted bincount over gt labels
(exactly reproducing segment_sum + masked mean -> scalar loss).

Device layout: voxels of a per-core chunk are split into G=9 groups of
GL contiguous voxels; SBUF tiles are [126, F] with partition r = g*14+c
holding channel c of voxel-group g (r traverses (g, c) lexicographically,
so DRAM views map to flat tiles). Channel sums over the 14 partitions of
each group are TensorE matmuls with a block-ones lhsT; slice k of a pack
uses lhsT_k [126, 108] with ones at [g*14+c, 9k+g], accumulating 12
slices into one PSUM bank [108, 512] so PSUM->SBUF copies and the output
DMA run at full partition utilization.

Inputs stream as fp16 (host-converted: halves HBM traffic; logits are
N(0,1) so fp16 quantization perturbs the final scalar by ~4e-6 relative).
Matmul operands are fp16 (1 col/cycle on PE vs 4 for fp32); PSUM
accumulation stays fp32 and the ZT/W/ZS outputs are returned as fp32.

Sharding: data-parallel over voxels, 8 cores, each core takes a
contiguous 1/8 slice of both batches. Scalar reduction happens on host.
"""

import numpy as np

for _p in ("/opt/trn_rl_repo", "/root/.axon_site/_ro/trn_rl_repo"):
    import sys

    if _p not in sys.path:
        sys.path.append(_p)

import concourse.bacc as bacc
import concourse.bass as bass
import concourse.tile as tile
from concourse import mybir
from concourse.bass_utils import run_bass_kernel_spmd

F32 = mybir.dt.float32
F16 = mybir.dt.float16
AF = mybir.ActivationFunctionType

B = 2
C = 14
N_TOT = 96 * 96 * 96          # 884736 voxels per batch
NCORES = 8
NC_VOX = N_TOT // NCORES      # 110592 voxels per core per batch
G = 9                         # voxel groups -> 126 = 9*14 used partitions
GL = NC_VOX // G              # 12288 voxels per group
SL = 512                      # matmul slice = one fp32 PSUM bank
K_PER_PACK = 12               # slices packed per PSUM bank (108 partitions)
PACK_F = SL * K_PER_PACK      # 6144 free-span per pack
N_PACKS = GL // PACK_F        # 2 packs per batch
QUARTERS = 2                  # loads per pack
Q_F = PACK_F // QUARTERS      # 3072 free-span per load
PACK_ROWS = G * K_PER_PACK    # 108
NQ = 3                        # ZT, W, ZS

_NC_CACHE = {}


def _build_nc():
    nc = bacc.Bacc("TRN2", target_bir_lowering=False, debug=False)

    s_dram = nc.dram_tensor("s", [B, C, NC_VOX], F16, kind="ExternalInput")
    t_dram = nc.dram_tensor("t", [B, C, NC_VOX], F16, kind="ExternalInput")
    # lhsT_k [126, 108]: ones at [g*14+c, 9k+g]
    ones_dram = nc.dram_tensor(
        "ones_blk", [126, K_PER_PACK, PACK_ROWS], F16, kind="ExternalInput"
    )
    # per (batch, pack): rows r=9k+g, then ZT|W|ZS, then 512 voxel cols
    out_dram = nc.dram_tensor(
        "zws", [B, N_PACKS, PACK_ROWS, NQ, SL], F32, kind="ExternalOutput"
    )

    s_ap = s_dram.ap()
    t_ap = t_dram.ap()
    out_ap = out_dram.ap()

    with tile.TileContext(nc) as tc:
        with (
            tc.tile_pool(name="singles", bufs=1) as singles,
            tc.tile_pool(name="io_s", bufs=3) as io_s,
            tc.tile_pool(name="io_t", bufs=3) as io_t,
            tc.tile_pool(name="es", bufs=3) as es_pool,
            tc.tile_pool(name="dd", bufs=3) as dd_pool,
            tc.tile_pool(name="et", bufs=6) as et_pool,
            tc.tile_pool(name="pp", bufs=6) as pp_pool,
            tc.tile_pool(name="psum", bufs=2, space="PSUM") as psum,
            tc.tile_pool(name="cop", bufs=2) as cop_pool,
        ):
            ones_t = singles.tile([126, K_PER_PACK, PACK_ROWS], F16)
            nc.sync.dma_start(out=ones_t[:], in_=ones_dram.ap())

            nsl = Q_F // SL
            H = Q_F // 2

            # The zs stream runs one quarter behind zt/wm: each quarter's
            # PE emission is (zt_j, wm_j, prev.zs_j) triples, which keeps
            # PSUM banks alternating (no same-bank back-to-back pair) while
            # zs always consumes an eS produced a full quarter earlier.
            packs = {}   # (b, p) -> dict(zt, wm, zs, done_ks, cop)
            prev = None  # dict(eS, zs_bank, ks, pack_key)

            def finish_pack(key):
                st = packs.pop(key)
                b_, p_ = key
                cop = cop_pool.tile([PACK_ROWS, NQ, SL], F32)
                nc.vector.tensor_copy(cop[:, 0, :], st["zt"][:])
                nc.vector.tensor_copy(cop[:, 1, :], st["wm"][:])
                nc.vector.tensor_copy(cop[:, 2, :], st["zs"][:])
                nc.sync.dma_start(out=out_ap[b_, p_], in_=cop[:])

            def emit_zs(pz, j):
                k = pz["ks"][j]
                nc.tensor.matmul(
                    pz["zs"][:, :], ones_t[:, k, :],
                    pz["eS"][:, j * SL : (j + 1) * SL],
                    start=(k == 0), stop=(k == K_PER_PACK - 1),
                )

            for b in range(B):
                # [C, NC_VOX] -> [G, C, GL]: partition row g*14+c <-> (g, c)
                sb = s_ap[b].rearrange("c (g f) -> g c f", g=G)
                tb = t_ap[b].rearrange("c (g f) -> g c f", g=G)
                for p in range(N_PACKS):
                    key = (b, p)
                    zt_bank = psum.tile([PACK_ROWS, SL], F32, tag="zt")
                    wm_bank = psum.tile([PACK_ROWS, SL], F32, tag="wm")
                    zs_bank = psum.tile([PACK_ROWS, SL], F32, tag="zs")
                    st = {"zt": zt_bank, "wm": wm_bank, "zs": zs_bank}
                    packs[key] = st
                    for q in range(QUARTERS):
                        f0 = p * PACK_F + q * Q_F
                        s_t = io_s.tile([126, Q_F], F16)
                        t_t = io_t.tile([126, Q_F], F16)
                        nc.sync.dma_start(
                            out=s_t[:], in_=sb[:, :, f0 : f0 + Q_F]
                        )
                        nc.sync.dma_start(
                            out=t_t[:], in_=tb[:, :, f0 : f0 + Q_F]
                        )
                        # et halves first so PE's zt/wm unblock early;
                        # eS last (only needed next quarter)
                        d = dd_pool.tile([126, Q_F], F16)
                        nc.vector.tensor_sub(d[:], t_t[:], s_t[:])
                        ets, pps = [], []
                        for h in range(2):
                            hc = slice(h * H, (h + 1) * H)
                            et = et_pool.tile([126, H], F16)
                            nc.scalar.activation(et[:], t_t[:, hc], AF.Exp)
                            pp = pp_pool.tile([126, H], F16)
                            nc.vector.tensor_mul(pp[:], et[:], d[:, hc])
                            ets.append(et)
                            pps.append(pp)
                        eS = es_pool.tile([126, Q_F], F16)
                        nc.scalar.activation(eS[:], s_t[:], AF.Exp)

                        for j in range(nsl):
                            k = q * nsl + j
                            lhsT = ones_t[:, k, :]
                            half, jj = ets[j // (nsl // 2)], j % (nsl // 2)
                            nc.tensor.matmul(
                                st["zt"][:, :], lhsT,
                                half[:, jj * SL : (jj + 1) * SL],
                                start=(k == 0), stop=(k == K_PER_PACK - 1),
                            )
                            half = pps[j // (nsl // 2)]
                            nc.tensor.matmul(
                                st["wm"][:, :], lhsT,
                                half[:, jj * SL : (jj + 1) * SL],
                                start=(k == 0), stop=(k == K_PER_PACK - 1),
                            )
                            if prev is not None:
                                emit_zs(prev, j)
                        if prev is not None and prev["final"]:
                            # prev pack's zs got its stop matmul above
                            finish_pack(prev["pack_key"])
                        prev = {
                            "eS": eS,
                            "zs": st["zs"],
                            "ks": [q * nsl + j for j in range(nsl)],
                            "pack_key": key,
                            "final": q == QUARTERS - 1,
                        }

            # drain the final quarter's zs and close remaining packs
            for j in range(nsl):
                emit_zs(prev, j)
            for key in list(packs):
                finish_pack(key)

    nc.compile()
    return nc


def _get_nc():
    if "nc" not in _NC_CACHE:
        _NC_CACHE["nc"] = _build_nc()
    return _NC_CACHE["nc"]


def _ones_blk():
    o = np.zeros((126, K_PER_PACK, PACK_ROWS), dtype=np.float16)
    r = np.arange(126)
    for k in range(K_PER_PACK):
        o[r, k, G * k + r // C] = 1.0
    return o


def kernel(preds_S, preds_T, gt_labels, _results_hook=None):
    S = np.asarray(preds_S, dtype=np.float16).reshape(B, C, N_TOT)
    T = np.asarray(preds_T, dtype=np.float16).reshape(B, C, N_TOT)
    labels = np.asarray(gt_labels).reshape(B, N_TOT)

    nc = _get_nc()
    ones = _ones_blk()
    in_maps = []
    for m in range(NCORES):
        sl = slice(m * NC_VOX, (m + 1) * NC_VOX)
        in_maps.append(
            {
                "s": np.ascontiguousarray(S[:, :, sl]),
                "t": np.ascontiguousarray(T[:, :, sl]),
                "ones_blk": ones,
            }
        )

    res = run_bass_kernel_spmd(nc, in_maps, list(range(NCORES)))
    if _results_hook is not None:
        _results_hook(res)

    # reassemble ZT/W/ZS into [B, N_TOT] voxel order:
    # out[b, p, 9k+g, q, v] <-> voxel (core m) m*NC_VOX + g*GL + p*PACK_F + k*SL + v
    fields = np.empty((NQ, B, N_TOT), dtype=np.float32)
    for m in range(NCORES):
        zws = res.results[m]["zws"]  # [B, N_PACKS, 108, 3, 512]
        a = zws.reshape(B, N_PACKS, K_PER_PACK, G, NQ, SL)
        # -> [NQ, B, G, N_PACKS, K_PER_PACK, SL] -> [NQ, B, NC_VOX]
        a = a.transpose(4, 0, 3, 1, 2, 5).reshape(NQ, B, NC_VOX)
        fields[:, :, m * NC_VOX : (m + 1) * NC_VOX] = a

    ZT, W, ZS = fields[0], fields[1], fields[2]
    kl = W / ZT + np.log(ZS) - np.log(ZT)

    # host finale: segment sums per (batch, class), masked mean, class 0 excluded
    loss = 0.0
    for b in range(B):
        lab = labels[b].astype(np.int64)
        sums = np.bincount(lab, weights=kl[b].astype(np.float64), minlength=C)
        counts = np.bincount(lab, minlength=C)
        terms = np.where(counts > 0, sums / (C * np.maximum(counts, 1)), 0.0)
        loss += terms[1:].sum()
    return np.float32(loss)
